# revision 5
# baseline (speedup 1.0000x reference)
"""Trainium2 Bass kernel for nn_ConfidenceLossV2 (segment_reduce, memory-bound).

Sharding: data-parallel over the batch dim — 8 batch items, one per NeuronCore.
Each core computes 4 partial scalars (segment-loss numerator/denominator and
recovery-loss numerator/denominator); the host sums them across cores and does
the two final divisions (the "psum of weighted sums and counts").

Per-core pipeline (pixel-major layout, feature grid r,q in [0,128)^2,
partition = r):
  - enc/dec are DMA'd with f32->bf16 cast into [128, 64c, 128q] tiles;
    x = enc-dec on DVE (bf16 2x mode), y = x^2 on ACT into y[128, 66*128]
    whose last two 128-col blocks hold ones and pos=(0<mask_i<0.5).
  - one-hot eq[p, s*128+q] = (seg[p,q]==s) via one DVE is_equal against a
    GPSIMD-iota tile.
  - segment sums: 64 matmuls, each contracting K=128 pixel-rows for a pair of
    pixel-columns (q, q+1): lhsT = eq columns pair [128,128], rhs = y pair
    [128,132], accumulated in PSUM [128,132]; a fold matmul adds the two
    64-row halves, then counts/pos_cnt/err sums are extracted.
  - recovery loss: per channel t=in*lt, d=out-t, dm=d*m01 on DVE, then ACT
    Square with accum_out -> per-partition sums; (mask<0.5), (mask>0) via
    tensor_scalar (the latter with accum_out giving sum(m) for free).
  - a final ones-matmul reduces the 4 per-partition columns to [1,4].
"""

import os
import sys

for _p in ("/opt/trn_rl_repo",):
    if _p not in sys.path and os.path.isdir(_p):
        sys.path.insert(0, _p)

import numpy as np

N_CORES = 8
C_IMG, H, W = 3, 512, 512
C_FEAT, HE, WE = 64, 128, 128
N_SEG = 64
NPIX = HE * WE  # 16384
PIX_FREE = (H * W) // 128  # 2048 free elems per partition at image res
WALL_COT = 0.5
MIN_FRAC = 0.01

_CACHE = {}


def _build():
    import concourse.bacc as bacc
    import concourse.bass as bass
    import concourse.tile as tile
    from concourse import mybir

    dt = mybir.dt
    BF = dt.bfloat16
    F32 = dt.float32
    Alu = mybir.AluOpType
    Act = mybir.ActivationFunctionType

    nc = bacc.Bacc("TRN2", target_bir_lowering=False, debug=False,
                   enable_asserts=False, num_devices=N_CORES)

    outputs_d = nc.dram_tensor("outputs", [C_IMG, H, W], F32, kind="ExternalInput").ap()
    inputs_d = nc.dram_tensor("inputs", [C_IMG, H, W], F32, kind="ExternalInput").ap()
    enc_d = nc.dram_tensor("enc1", [C_FEAT, HE, WE], F32, kind="ExternalInput").ap()
    dec_d = nc.dram_tensor("dec1", [C_FEAT, HE, WE], F32, kind="ExternalInput").ap()
    masks_d = nc.dram_tensor("masks", [H, W], F32, kind="ExternalInput").ap()
    segs_d = nc.dram_tensor("segs", [H, W], dt.int32, kind="ExternalInput").ap()
    part_d = nc.dram_tensor("partials", [1, 4], F32, kind="ExternalOutput").ap()

    def sub_ap(t, extra_off, dims):
        # manual AP view of a tile: dims = [[step, count], ...] free dims
        return bass.AP(tensor=t.tensor, offset=t.offset + extra_off,
                       ap=[list(t.ap[0])] + [list(d) for d in dims])

    with tile.TileContext(nc) as tc:
        with (
            tc.tile_pool(name="big", bufs=1) as big,
            tc.tile_pool(name="scr", bufs=3) as scrp,
            tc.tile_pool(name="small", bufs=1) as small,
            tc.tile_pool(name="ps", bufs=1, space="PSUM") as psp,
        ):
            # ---- tiles ----
            seg_rows = big.tile([128, W], dt.int32)        # every 4th image row
            segf = small.tile([128, WE], BF)               # seg ids at feature res
            iota_t = big.tile([128, N_SEG, WE], BF)        # value = s at [p, s, q]
            eq = big.tile([128, N_SEG, WE], BF)            # one-hot, f = s*128+q
            Eb = big.tile([128, C_FEAT, WE], BF)
            Db = big.tile([128, C_FEAT, WE], BF)
            xt = big.tile([128, C_FEAT, WE], BF)
            yt = big.tile([128, C_FEAT + 2, WE], BF)       # squares | ones | pos
            Mt = big.tile([128, PIX_FREE], BF)
            OT = big.tile([128, C_IMG, PIX_FREE], BF)
            IT = big.tile([128, C_IMG, PIX_FREE], BF)
            lt = big.tile([128, PIX_FREE], BF)
            m01 = big.tile([128, PIX_FREE], BF)
            posg = small.tile([128, WE], BF)
            racc = small.tile([128, 4], F32)
            rhsf = small.tile([128, 4], F32)
            ones128 = small.tile([128, 1], F32)
            out_sb = small.tile([1, 4], F32)

            err_sum = small.tile([N_SEG, 1], F32)
            safe = small.tile([N_SEG, 1], F32)
            rsafe = small.tile([N_SEG, 1], F32)
            mean_err = small.tile([N_SEG, 1], F32)
            ratio = small.tile([N_SEG, 1], F32)
            validt = small.tile([N_SEG, 1], F32)
            pflag = small.tile([N_SEG, 1], F32)

            psumA = psp.tile([N_SEG, C_FEAT + 2], F32)
            psumF = psp.tile([1, 4], F32)

            # ---- DMAs (issue order ~ priority: cheap deps first) ----
            # segs: only every 4th image row is needed (nearest downsample)
            nc.sync.dma_start(out=seg_rows,
                              in_=segs_d.rearrange("(p r) w -> p r w", r=4)[:, 0, :])
            # f32 -> bf16 casting loads must use gpsimd (SWDGE)
            nc.gpsimd.dma_start(out=Mt, in_=masks_d.rearrange("(p r) w -> p (r w)", r=4))
            nc.gpsimd.dma_start(out=Eb, in_=enc_d.rearrange("c r q -> r c q"))
            nc.gpsimd.dma_start(out=Db, in_=dec_d.rearrange("c r q -> r c q"))
            for c in range(C_IMG):
                nc.gpsimd.dma_start(
                    out=OT[:, c, :],
                    in_=outputs_d[c].rearrange("(p r) w -> p (r w)", r=4))
                nc.gpsimd.dma_start(
                    out=IT[:, c, :],
                    in_=inputs_d[c].rearrange("(p r) w -> p (r w)", r=4))

            # iota over [s, q]: value = s (channel_multiplier=0)
            nc.gpsimd.iota(iota_t, pattern=[[1, N_SEG], [0, WE]], base=0,
                           channel_multiplier=0,
                           allow_small_or_imprecise_dtypes=True)

            # ---- one-hot build ----
            # segf = bf16(seg_rows[:, ::4])
            nc.vector.tensor_copy(out=segf, in_=sub_ap(seg_rows, 0, [[4, WE]]))
            # eq[p, s, q] = (segf[p, q] == s)
            segf_b = sub_ap(segf, 0, [[0, N_SEG], [1, WE]])
            nc.vector.tensor_tensor(out=eq, in0=segf_b, in1=iota_t, op=Alu.is_equal)

            # ---- enc/dec -> y ----
            nc.vector.tensor_tensor(out=xt, in0=Eb, in1=Db, op=Alu.subtract)
            nc.scalar.activation(out=yt[:, 0:C_FEAT, :], in_=xt, func=Act.Square)
            nc.vector.memset(yt[:, C_FEAT, :], 1.0)
            # pos = (mask_i > 0) * (mask_i < 0.5); mask_i = Mt[:, ::4] (128 vals)
            mask_i = sub_ap(Mt, 0, [[4, WE]])
            nc.vector.tensor_scalar(out=posg, in0=mask_i, scalar1=0.0, scalar2=None,
                                    op0=Alu.is_gt)
            nc.vector.scalar_tensor_tensor(out=yt[:, C_FEAT + 1, :], in0=mask_i,
                                           scalar=WALL_COT, in1=posg,
                                           op0=Alu.is_lt, op1=Alu.mult)

            # ---- recovery loss elementwise ----
            nc.vector.memset(rhsf, 0.0)
            nc.vector.memset(racc, 0.0)
            nc.vector.tensor_scalar(out=lt, in0=Mt, scalar1=WALL_COT, scalar2=None,
                                    op0=Alu.is_lt)
            # op1 is the accumulation op when accum_out is given
            nc.vector.tensor_scalar(out=m01, in0=Mt, scalar1=0.0, scalar2=None,
                                    op0=Alu.is_gt, op1=Alu.add,
                                    accum_out=rhsf[:, 3:4])
            for c in range(C_IMG):
                ttile = scrp.tile([128, PIX_FREE], BF, tag="t")
                dtile = scrp.tile([128, PIX_FREE], BF, tag="d")
                dmt = scrp.tile([128, PIX_FREE], BF, tag="dm")
                sq = scrp.tile([128, PIX_FREE], BF, tag="sq")
                nc.vector.tensor_tensor(out=ttile, in0=IT[:, c, :], in1=lt, op=Alu.mult)
                nc.vector.tensor_tensor(out=dtile, in0=OT[:, c, :], in1=ttile,
                                        op=Alu.subtract)
                nc.vector.tensor_tensor(out=dmt, in0=dtile, in1=m01, op=Alu.mult)
                nc.scalar.activation(out=sq, in_=dmt, func=Act.Square,
                                     accum_out=racc[:, c:c + 1])
            nc.vector.tensor_tensor(out=racc[:, 3:4], in0=racc[:, 0:1],
                                    in1=racc[:, 1:2], op=Alu.add)
            nc.vector.tensor_tensor(out=rhsf[:, 2:3], in0=racc[:, 3:4],
                                    in1=racc[:, 2:3], op=Alu.add)

            # ---- segment-sum matmuls: one K=128 contraction per pixel column ----
            # lhsT = eq[:, :, q] (one-hot columns), rhs = yt[:, :, q]
            # (64 squares | 1 | pos), accumulated into psumA[s, c|ones|pos].
            for q in range(WE):
                lhsT = sub_ap(eq, q, [[WE, N_SEG]])
                rhs = sub_ap(yt, q, [[WE, C_FEAT + 2]])
                nc.tensor.matmul(psumA, lhsT, rhs, start=(q == 0),
                                 stop=(q == WE - 1))

            # ---- per-segment selection ----
            nc.vector.tensor_reduce(out=err_sum, in_=psumA[0:N_SEG, 0:C_FEAT],
                                    axis=mybir.AxisListType.X, op=Alu.add)
            counts = psumA[0:N_SEG, C_FEAT:C_FEAT + 1]
            poscnt = psumA[0:N_SEG, C_FEAT + 1:C_FEAT + 2]
            nc.vector.tensor_scalar(out=safe, in0=counts, scalar1=1.0, scalar2=None,
                                    op0=Alu.max)
            nc.vector.reciprocal(out=rsafe, in_=safe)
            nc.vector.scalar_tensor_tensor(out=mean_err, in0=err_sum,
                                           scalar=1.0 / C_FEAT, in1=rsafe,
                                           op0=Alu.mult, op1=Alu.mult)
            nc.vector.tensor_tensor(out=ratio, in0=poscnt, in1=rsafe, op=Alu.mult)
            thr_cnt = float(np.float32(MIN_FRAC)) * NPIX
            nc.vector.tensor_scalar(out=validt, in0=counts, scalar1=thr_cnt,
                                    scalar2=None, op0=Alu.is_ge)
            nc.vector.tensor_scalar(out=pflag, in0=ratio,
                                    scalar1=float(np.float32(MIN_FRAC)),
                                    scalar2=None, op0=Alu.is_gt)
            nc.vector.tensor_tensor(out=rhsf[0:N_SEG, 1:2], in0=validt, in1=pflag,
                                    op=Alu.mult)
            nc.vector.tensor_tensor(out=rhsf[0:N_SEG, 0:1], in0=mean_err,
                                    in1=rhsf[0:N_SEG, 1:2], op=Alu.mult)

            # ---- final partition reduction and output ----
            nc.vector.memset(ones128, 1.0)
            nc.tensor.matmul(psumF, ones128, rhsf, start=True, stop=True)
            nc.vector.tensor_copy(out=out_sb, in_=psumF)
            nc.sync.dma_start(out=part_d, in_=out_sb)

    nc.compile()
    return nc


def _get_nc():
    if "nc" not in _CACHE:
        _CACHE["nc"] = _build()
    return _CACHE["nc"]


def kernel(outputs, inputs, enc1, dec1, masks, segs, confidence=None,
           iteration=None, epoch=None, **_unused):
    from concourse import bass_utils

    outputs = np.asarray(outputs, dtype=np.float32)
    inputs = np.asarray(inputs, dtype=np.float32)
    enc1 = np.asarray(enc1, dtype=np.float32)
    dec1 = np.asarray(dec1, dtype=np.float32)
    masks = np.asarray(masks, dtype=np.float32)
    segs = np.asarray(segs, dtype=np.int32)

    nc = _get_nc()
    in_maps = []
    for b in range(N_CORES):
        in_maps.append({
            "outputs": np.ascontiguousarray(outputs[b]),
            "inputs": np.ascontiguousarray(inputs[b]),
            "enc1": np.ascontiguousarray(enc1[b]),
            "dec1": np.ascontiguousarray(dec1[b]),
            "masks": np.ascontiguousarray(masks[b, 0]),
            "segs": np.ascontiguousarray(segs[b, 0]),
        })

    trace = bool(int(os.environ.get("KERNEL_TRACE", "0")))
    res = bass_utils.run_bass_kernel_spmd(nc, in_maps,
                                          core_ids=list(range(N_CORES)),
                                          trace=trace)
    _CACHE["last_result"] = res

    parts = np.stack([res.results[b]["partials"].reshape(4) for b in range(N_CORES)])
    f32 = np.float32
    num1 = f32(parts[:, 0].astype(np.float32).sum(dtype=np.float32))
    den1 = f32(parts[:, 1].astype(np.float32).sum(dtype=np.float32))
    num2 = f32(parts[:, 2].astype(np.float32).sum(dtype=np.float32))
    den2 = f32(parts[:, 3].astype(np.float32).sum(dtype=np.float32))
    flat_pos_mean = f32(num1 / max(den1, f32(1.0)))
    loss_recov = f32(num2 / max(den2, f32(1.0)))
    return np.asarray(f32(loss_recov + flat_pos_mean))
